# revision 3
# baseline (speedup 1.0000x reference)
"""Trainium2 Bass kernel for Converse2D (N,C,H,W)=(4,64,128,128), scale=2, pad=4.

Math: the whole op collapses to per-channel spectral masks applied between a
forward 136x136 2D DFT (circular pad folded into the DFT matrix) and four
cropped inverse DFTs (one per output polyphase), all realized as PE matmuls:

  out[2u+rr, 2v+rc] interleaved from  O_pair(rr) = Br4 @ (Bc @ (Psi ⊙ Xp))^T

with Hermitian folding along one frequency axis (69+67 rows instead of 136)
so every matmul contraction is <=128 and the column flip folds into masks.

Sharding: channels 8 per core x 8 cores; each core does 32 (n,c) planes.
"""
import sys
import numpy as np

if "/opt/trn_rl_repo" not in sys.path:
    sys.path.insert(0, "/opt/trn_rl_repo")

SCALE, PAD, EPS, KK = 2, 4, 1e-5, 5
N, C, H, W = 4, 64, 128, 128
h = H + 2 * PAD          # 136
hs = h * SCALE           # 272
NCH = 8                  # channels per core
NP = N * NCH             # planes per core (32)
F32 = np.float32

_cache = {}


# ---------------------------------------------------------------- host math --
def _constants():
    if "consts" in _cache:
        return _cache["consts"]
    i = np.arange(h)
    F136 = np.exp(-2j * np.pi * np.outer(i, i) / h)
    P = np.zeros((h, W))
    for r in range(h):
        P[r, (r - PAD) % W] = 1.0
    Ac = F136 @ P                                            # [136,128]
    t = np.arange(128) + PAD
    k = np.arange(h)
    Bc = np.exp(2j * np.pi * np.outer(t, k) / h)             # [128,136]
    kc2 = np.arange(1, 68)
    B2c = Bc[:, (h - kc2) % h]                               # [128,67]
    Bc69 = Bc[:, 0:69]
    B4b, B4s = Bc[:, 0:128], Bc[:, 128:136]
    cst = dict(
        cs1=np.concatenate([Ac.real.T, Ac.imag.T], axis=1),
        cs2=np.concatenate([Ac[0:69].real.T, Ac[0:69].imag.T,
                            Ac[1:68].real.T, Ac[1:68].imag.T], axis=1),
        r3a=np.concatenate([Bc69.real.T, Bc69.imag.T], axis=1),
        r3b=np.concatenate([-Bc69.imag.T, Bc69.real.T], axis=1),
        r3c=np.concatenate([B2c.real.T, B2c.imag.T], axis=1),
        r3d=np.concatenate([-B2c.imag.T, B2c.real.T], axis=1),
        s4a=np.concatenate([B4b.real.T, -B4b.imag.T, B4b.imag.T], axis=1),
        s4b=np.concatenate([B4s.real.T, -B4s.imag.T, B4s.imag.T], axis=1),
    )
    cst = {k: np.ascontiguousarray(v, F32) for k, v in cst.items()}
    _cache["consts"] = cst
    return cst


def _masks(weight, bias):
    """-> m1 [C,69,544], m2 [C,67,544] fp32."""
    Cn = C
    w = np.asarray(weight, np.float64).reshape(Cn, KK, KK)
    b = np.asarray(bias, np.float64).reshape(Cn)
    be = 1.0 / (1.0 + np.exp(-(b - 9.0))) + EPS
    otf = np.zeros((Cn, hs, hs), np.complex128)
    otf[:, :KK, :KK] = w
    otf = np.roll(otf, (-(KK // 2), -(KK // 2)), axis=(1, 2))
    FB = np.fft.fft2(otf)
    FBC = np.conj(FB)
    F2B = np.abs(FB) ** 2
    kkk = np.arange(hs)
    e = 1.0 + np.exp(-2j * np.pi * kkk / hs)
    E = np.outer(e, e)

    def qmean(a):
        return 0.25 * (a[..., :h, :h] + a[..., :h, h:] + a[..., h:, :h] + a[..., h:, h:])

    Q = qmean(F2B)
    G = qmean(F2B + be[:, None, None] * FB * E[None])
    Hs = G / (Q + be[:, None, None])
    Phi = (FBC + be[:, None, None] * E[None]
           - FBC * np.tile(Hs, (1, 2, 2))) / be[:, None, None]

    kr = np.arange(h)
    m1 = np.zeros((Cn, 69, 544), F32)
    m2 = np.zeros((Cn, 67, 544), F32)
    for rr in (0, 1):
        psis = []
        for rc in (0, 1):
            acc = np.zeros((Cn, h, h), np.complex128)
            for qr in (0, 1):
                for qc in (0, 1):
                    sgn = (-1.0) ** (qr * rr + qc * rc)
                    acc += sgn * Phi[:, qr * h:(qr + 1) * h, qc * h:(qc + 1) * h]
            phase = np.exp(2j * np.pi * (np.outer(kr * rr, np.ones(h)) +
                                          np.outer(np.ones(h), kr * rc)) / hs)
            psis.append(0.25 * phase[None] * acc / (h * h))
        Mpair = (psis[0] + 1j * psis[1]).transpose(0, 2, 1)    # [C, kc, kr]
        kc = np.arange(1, 68)
        M1p, M2p = Mpair[:, 0:69, :], Mpair[:, (h - kc) % h, :]
        base = rr * 272
        m1[:, :, base:base + 136] = M1p.real
        m1[:, :, base + 136:base + 272] = M1p.imag
        m2[:, :, base:base + 136] = M2p.real
        m2[:, :, base + 136:base + 272] = M2p.imag
    return m1, m2


# ------------------------------------------------------------- bass program --
def _program(nplanes=NP):
    key = ("prog", nplanes)
    if key in _cache:
        return _cache[key]
    import concourse.bass as bass
    import concourse.tile as tile
    from concourse import bacc, mybir
    from concourse.alu_op_type import AluOpType

    f32 = mybir.dt.float32
    f32r = mybir.dt.float32r
    nc = bacc.Bacc("TRN2", target_bir_lowering=False, debug=False,
                   enable_asserts=False, num_devices=8)

    xs_d = nc.dram_tensor("xs", (nplanes, 128, 128), f32, kind="ExternalInput").ap()
    m1_d = nc.dram_tensor("m1", (69, NCH * 544), f32, kind="ExternalInput").ap()
    m2_d = nc.dram_tensor("m2", (67, NCH * 544), f32, kind="ExternalInput").ap()
    cd = {}
    for nm, shp in [("cs1", (128, 272)), ("cs2", (128, 272)),
                    ("r3a", (69, 256)), ("r3b", (69, 256)),
                    ("r3c", (67, 256)), ("r3d", (67, 256)),
                    ("s4a", (128, 384)), ("s4b", (8, 384))]:
        cd[nm] = nc.dram_tensor(nm, shp, f32, kind="ExternalInput").ap()
    out_d = nc.dram_tensor("out", (nplanes, 256, 256), f32, kind="ExternalOutput").ap()

    r = lambda ap: ap.bitcast(f32r)

    with tile.TileContext(nc) as tc:
        with (
            tc.tile_pool(name="consts", bufs=1) as cpool,
            tc.tile_pool(name="xin", bufs=3) as xpool,
            tc.tile_pool(name="tt", bufs=2) as tpool,
            tc.tile_pool(name="uu", bufs=2) as upool,
            tc.tile_pool(name="mt", bufs=4) as mpool,
            tc.tile_pool(name="vv", bufs=2) as vpool,
            tc.tile_pool(name="ob", bufs=4) as opool,
            tc.tile_pool(name="psY", bufs=2, space="PSUM") as psY,
            tc.tile_pool(name="psSa", bufs=1, space="PSUM") as psSa,
            tc.tile_pool(name="psSb", bufs=1, space="PSUM") as psSb,
            tc.tile_pool(name="psV", bufs=2, space="PSUM") as psV,
            tc.tile_pool(name="psO", bufs=2, space="PSUM") as psO,
        ):
            # resident constants
            csb = {}
            for nm in cd:
                t = cpool.tile(list(cd[nm].shape), f32r, tag=nm)
                nc.sync.dma_start(t[:], cd[nm][:].bitcast(f32r))
                csb[nm] = t
            m1sb = cpool.tile([69, NCH * 544], f32, tag="m1sb")
            nc.sync.dma_start(m1sb[:], m1_d[:])
            m2sb = cpool.tile([67, NCH * 544], f32, tag="m2sb")
            nc.sync.dma_start(m2sb[:], m2_d[:])

            for p in range(nplanes):
                c = p // N  # local channel index
                xt = xpool.tile([128, 128], f32r, tag="x")
                nc.sync.dma_start(xt[:], xs_d[p].bitcast(f32r))

                # S1: Y1 = x^T @ [AcrT|AciT]  -> PSUM [128,272]
                Y1 = psY.tile([128, 272], f32, tag="y")
                nc.tensor.matmul(Y1[:], r(xt[:]), r(csb["cs1"][:]),
                                 start=True, stop=True)

                # T-stage (ACT): T1=[Yr|Yr0|Yi|Yi0], T2=[-Yi|-Yi0|Yr|Yr0]
                T1 = tpool.tile([128, 274], f32r, tag="t1")
                T2 = tpool.tile([128, 274], f32r, tag="t2")
                nc.scalar.copy(T1[:, 0:136], Y1[:, 0:136])
                nc.scalar.copy(T1[:, 136:137], Y1[:, 0:1])
                nc.scalar.copy(T1[:, 137:273], Y1[:, 136:272])
                nc.scalar.copy(T1[:, 273:274], Y1[:, 136:137])
                nc.scalar.mul(T2[:, 0:136], Y1[:, 136:272], -1.0)
                nc.scalar.mul(T2[:, 136:137], Y1[:, 136:137], -1.0)
                nc.scalar.copy(T2[:, 137:273], Y1[:, 0:136])
                nc.scalar.copy(T2[:, 273:274], Y1[:, 0:1])

                # S2: Sa[69,274], Sb[67,274]
                Sa = psSa.tile([69, 274], f32, tag="sa")
                Sb = psSb.tile([67, 274], f32, tag="sb")
                nc.tensor.matmul(Sa[:], r(csb["cs2"][:, 0:69]), r(T1[:]), start=True, stop=False)
                nc.tensor.matmul(Sa[:], r(csb["cs2"][:, 69:138]), r(T2[:]), start=False, stop=True)
                nc.tensor.matmul(Sb[:], r(csb["cs2"][:, 138:205]), r(T1[:]), start=True, stop=False)
                nc.tensor.matmul(Sb[:], r(csb["cs2"][:, 205:272]), r(T2[:]), start=False, stop=True)

                # masks -> U tiles (pair-batched on free axis)
                U1r = upool.tile([69, 272], f32r, tag="u1r")
                U1i = upool.tile([69, 272], f32r, tag="u1i")
                U2r = upool.tile([67, 272], f32r, tag="u2r")
                U2i = upool.tile([67, 272], f32r, tag="u2i")
                SaR, SaI = Sa[:, 0:136], Sa[:, 137:273]
                SfR, SfI = Sb[:, 136:0:-1], Sb[:, 273:137:-1]
                for rr in (0, 1):
                    mb = c * 544 + rr * 272
                    M1r = m1sb[:, mb:mb + 136]
                    M1i = m1sb[:, mb + 136:mb + 272]
                    M2r = m2sb[:, mb:mb + 136]
                    M2i = m2sb[:, mb + 136:mb + 272]
                    sl = slice(rr * 136, rr * 136 + 136)
                    ta = mpool.tile([69, 136], f32, tag="ta")
                    tb = mpool.tile([69, 136], f32, tag="tb")
                    nc.vector.tensor_mul(ta[:], M1r, SaR)
                    nc.vector.tensor_mul(tb[:], M1i, SaI)
                    nc.gpsimd.tensor_tensor(U1r[:, sl], ta[:], tb[:], AluOpType.subtract)
                    tc2 = mpool.tile([69, 136], f32, tag="ta")
                    td = mpool.tile([69, 136], f32, tag="tb")
                    nc.vector.tensor_mul(tc2[:], M1r, SaI)
                    nc.vector.tensor_mul(td[:], M1i, SaR)
                    nc.gpsimd.tensor_tensor(U1i[:, sl], tc2[:], td[:], AluOpType.add)
                    te = mpool.tile([67, 136], f32, tag="tc")
                    tf = mpool.tile([67, 136], f32, tag="td")
                    nc.vector.tensor_mul(te[:], M2r, SfR)
                    nc.vector.tensor_mul(tf[:], M2i, SfI)
                    nc.gpsimd.tensor_tensor(U2r[:, sl], te[:], tf[:], AluOpType.add)
                    tg = mpool.tile([67, 136], f32, tag="tc")
                    th = mpool.tile([67, 136], f32, tag="td")
                    nc.vector.tensor_mul(tg[:], M2i, SfR)
                    nc.vector.tensor_mul(th[:], M2r, SfI)
                    nc.gpsimd.tensor_tensor(U2i[:, sl], tg[:], th[:], AluOpType.subtract)

                # S3' (U as stationary): Vt_p = [Vr^T | Vi^T] per pair
                Vts = vpool.tile([128, 512], f32r, tag="vts")
                Vsm = vpool.tile([8, 512], f32r, tag="vsm")
                for rr in (0, 1):
                    Vt = psV.tile([128, 512], f32, tag="vt")
                    big = slice(rr * 136, rr * 136 + 128)
                    sml = slice(rr * 136 + 128, rr * 136 + 136)
                    nc.tensor.matmul(Vt[:, 0:256], r(U1r[:, big]), r(csb["r3a"][:]), start=True, stop=False)
                    nc.tensor.matmul(Vt[:, 0:256], r(U1i[:, big]), r(csb["r3b"][:]), start=False, stop=False)
                    nc.tensor.matmul(Vt[:, 0:256], r(U2r[:, big]), r(csb["r3c"][:]), start=False, stop=False)
                    nc.tensor.matmul(Vt[:, 0:256], r(U2i[:, big]), r(csb["r3d"][:]), start=False, stop=True)
                    nc.tensor.matmul(Vt[0:8, 256:512], r(U1r[:, sml]), r(csb["r3a"][:]), start=True, stop=False)
                    nc.tensor.matmul(Vt[0:8, 256:512], r(U1i[:, sml]), r(csb["r3b"][:]), start=False, stop=False)
                    nc.tensor.matmul(Vt[0:8, 256:512], r(U2r[:, sml]), r(csb["r3c"][:]), start=False, stop=False)
                    nc.tensor.matmul(Vt[0:8, 256:512], r(U2i[:, sml]), r(csb["r3d"][:]), start=False, stop=True)
                    nc.scalar.copy(Vts[:, rr * 256:rr * 256 + 256], Vt[:, 0:256])
                    nc.scalar.copy(Vsm[:, rr * 256:rr * 256 + 256], Vt[0:8, 256:512])

                # S4: O = [Or0|Or1|Oi0|Oi1]  [128,512]
                O = psO.tile([128, 512], f32, tag="o")
                vb = Vts[:].rearrange("p (pr hf e) -> p pr hf e", pr=2, hf=2, e=128)
                vs = Vsm[:].rearrange("p (pr hf e) -> p pr hf e", pr=2, hf=2, e=128)
                vbR, vbI = vb[:, :, 0, :], vb[:, :, 1, :]
                vsR, vsI = vs[:, :, 0, :], vs[:, :, 1, :]
                s4a, s4b = csb["s4a"], csb["s4b"]
                nc.tensor.matmul(O[:, 0:256], r(s4a[:, 0:128]), r(vbR), start=True, stop=False)
                nc.tensor.matmul(O[:, 0:256], r(s4a[:, 128:256]), r(vbI), start=False, stop=False)
                nc.tensor.matmul(O[:, 0:256], r(s4b[:, 0:128]), r(vsR), start=False, stop=False)
                nc.tensor.matmul(O[:, 0:256], r(s4b[:, 128:256]), r(vsI), start=False, stop=True)
                nc.tensor.matmul(O[:, 256:512], r(s4a[:, 0:128]), r(vbI), start=True, stop=False)
                nc.tensor.matmul(O[:, 256:512], r(s4a[:, 256:384]), r(vbR), start=False, stop=False)
                nc.tensor.matmul(O[:, 256:512], r(s4b[:, 0:128]), r(vsI), start=False, stop=False)
                nc.tensor.matmul(O[:, 256:512], r(s4b[:, 256:384]), r(vsR), start=False, stop=True)

                # final interleave (DVE strided) + DMA out
                ob0 = opool.tile([128, 256], f32, tag="osb")
                ob1 = opool.tile([128, 256], f32, tag="osb")
                o0 = ob0[:].rearrange("p (e two) -> p e two", two=2)
                o1 = ob1[:].rearrange("p (e two) -> p e two", two=2)
                nc.vector.tensor_copy(o0[:, :, 0], O[:, 0:128])
                nc.vector.tensor_copy(o0[:, :, 1], O[:, 256:384])
                nc.vector.tensor_copy(o1[:, :, 0], O[:, 128:256])
                nc.vector.tensor_copy(o1[:, :, 1], O[:, 384:512])
                nc.sync.dma_start(out_d[p, 0::2, :], ob0[:])
                nc.sync.dma_start(out_d[p, 1::2, :], ob1[:])

    nc.compile()
    _cache[key] = nc
    return nc


# ------------------------------------------------------------------ runner --
def _in_maps(x, weight, bias):
    cst = _constants()
    m1, m2 = _masks(weight, bias)
    x = np.ascontiguousarray(np.asarray(x, F32))
    maps = []
    for k in range(8):
        ck = slice(NCH * k, NCH * k + NCH)
        maps.append({
            "xs": np.ascontiguousarray(
                x[:, ck].transpose(1, 0, 2, 3).reshape(NP, 128, 128)),
            "m1": np.ascontiguousarray(m1[ck].transpose(1, 0, 2).reshape(69, NCH * 544)),
            "m2": np.ascontiguousarray(m2[ck].transpose(1, 0, 2).reshape(67, NCH * 544)),
            **cst,
        })
    return maps


def run(x, weight, bias, trace=False):
    from concourse import bass_utils
    nc = _program()
    maps = _in_maps(x, weight, bias)
    res = bass_utils.run_bass_kernel_spmd(nc, maps, core_ids=list(range(8)),
                                          trace=trace)
    out = np.empty((N, C, 256, 256), F32)
    for k in range(8):
        out[:, NCH * k:NCH * k + NCH] = (
            res.results[k]["out"].reshape(NCH, N, 256, 256).transpose(1, 0, 2, 3))
    return out, res


def kernel(x, weight, bias):
    out, _ = run(x, weight, bias, trace=False)
    return out
